# revision 46
# baseline (speedup 1.0000x reference)
"""Trainium2 Bass kernel for the pre-norm transformer block (nn_Block_54511724920843).

Sharding: data-parallel over the batch axis b (8 groups of 1024 tokens, one per
NeuronCore). Weights replicated.

v2 design notes (vs the earlier baseline):
- The first collective of a NEFF execution costs ~65us (ncfw warmup) while
  later ones cost ~5us. A throwaway AllReduce is fired at t=0 to absorb the
  warmup concurrently with input DMA + BN1 stats.
- BN1 statistics are computed REDUNDANTLY on every core from a replicated
  bf16 copy of the full x (DVE bn_stats over [128, 8192] tiles, ~35us),
  removing the BN1 AllReduce from the critical path entirely. Only BN2
  (which depends on the attention output) uses a (now warm) AllReduce.
- All matmuls run in bf16 (same PE cycle rate as f32r at N>=256, half the
  SBUF, FWL weight loads, lower power -> less HAM throttling).
- Softmax: scoresT[k, q] with exp fused on ScalarE (mask bias + 1/8
  scale); head PAIRS issue their two K=64 score matmuls on disjoint PE
  row strips (base partitions 0/64) so they run concurrently.
  Denominators ride row 64 of the AV matmul via a ones-column appended
  to v; 1/denom runs on the DVE (reciprocal_approx_accurate, no ACT
  table switches), broadcast back with tiny ones-matmuls. The output
  projection contracts over head-pair blocks (K=128) after DMA-shifting
  odd heads' slices to partitions 64-127 mid-attention.
- V bias is folded into the output-projection bias host-side
  (bo2 = bo + bv @ Wo, exact because softmax rows sum to 1).
"""
import sys

sys.path.insert(0, "/opt/trn_rl_repo")

import numpy as np
import ml_dtypes

import concourse.bass as bass
import concourse.tile as tile
import concourse.mybir as mybir
from concourse import bacc
from concourse.bass_utils import run_bass_kernel_spmd

F32 = mybir.dt.float32
F32R = mybir.dt.float32r
BF16 = mybir.dt.bfloat16
AF = mybir.ActivationFunctionType
ALU = mybir.AluOpType

N_CORES = 8
B, GS, ED = 8, 1024, 512
H = 8
DH = ED // H            # 64
TOK = GS                # tokens per core
NT = ED // 128          # 4 feature tiles
NH = ED * 4 // 128      # 16 hidden tiles
NC_TOK = TOK // 128     # 8 token chunks
EPS = 1e-5
N_TOTAL = B * GS        # 8192


def _rstd_rsqrt(nc, small, name, var_ap, n_col):
    """rstd = sqrt(1/(var + eps)) — DVE exact reciprocal (tiny) + ACT Sqrt."""
    vpe = small.tile([128, n_col], F32, tag=f"vpe_{name}", name=f"vpe_{name}")
    nc.vector.tensor_scalar(out=vpe, in0=var_ap, scalar1=EPS, scalar2=None,
                            op0=ALU.add)
    rec = small.tile([128, n_col], F32, tag=f"rec_{name}", name=f"rec_{name}")
    nc.vector.reciprocal(out=rec, in_=vpe)
    rstd = small.tile([128, n_col], F32, tag=f"rst_{name}", name=f"rst_{name}")
    nc.scalar.activation(out=rstd, in_=rec, func=AF.Sqrt)
    return rstd


def _bn_partial(nc, small, name, t, x_tile, local):
    """Per-tile local (sum, sumsq)*TOK into local[:, 2t:2t+2] (for the AR)."""
    st = small.tile([128, 2, 6], F32, tag=f"st_{name}", name=f"st_{name}")
    nc.vector.bn_stats(out=st[:, 0, :], in_=x_tile[:, 0:512])
    nc.vector.bn_stats(out=st[:, 1, :], in_=x_tile[:, 512:1024])
    mv = small.tile([128, 2], F32, tag=f"mv_{name}", name=f"mv_{name}")
    nc.vector.bn_aggr(out=mv, in_=st)
    nc.vector.tensor_scalar(
        out=local[:, 2 * t : 2 * t + 1], in0=mv[:, 0:1],
        scalar1=float(TOK), scalar2=None, op0=ALU.mult)
    msq = small.tile([128, 1], F32, tag=f"msq_{name}", name=f"msq_{name}")
    nc.vector.tensor_mul(out=msq, in0=mv[:, 0:1], in1=mv[:, 0:1])
    vps = small.tile([128, 1], F32, tag=f"vps_{name}", name=f"vps_{name}")
    nc.vector.tensor_add(out=vps, in0=mv[:, 1:2], in1=msq)
    nc.vector.tensor_scalar(
        out=local[:, 2 * t + 1 : 2 * t + 2], in0=vps,
        scalar1=float(TOK), scalar2=None, op0=ALU.mult)


def _bn_device(nc, pools, g_sb, be_sb, name, collectives, local):
    """Global BN scale/shift from per-core partial stats via one AllReduce."""
    small, statsp, dram = pools
    bounce_in = dram.tile([128, 8], F32, tag=f"bi_{name}", name=f"bi_{name}")
    bounce_out = dram.tile([128, 8], F32, tag=f"bo_{name}", name=f"bo_{name}")
    nc.scalar.dma_start(out=bounce_in, in_=local)
    if collectives:
        nc.gpsimd.collective_compute(
            "AllReduce", ALU.add,
            replica_groups=[list(range(N_CORES))],
            ins=[bounce_in[:]], outs=[bounce_out[:]])
    else:
        nc.scalar.dma_start(out=bounce_out, in_=bounce_in[:])
    glob = statsp.tile([128, 8], F32, tag=f"gl_{name}", name=f"gl_{name}")
    nc.scalar.dma_start(out=glob, in_=bounce_out)

    s_all = statsp.tile([128, 4], F32, tag=f"s_{name}", name=f"s_{name}")
    t_all = statsp.tile([128, 4], F32, tag=f"t_{name}", name=f"t_{name}")
    inv_n = 1.0 / float(N_TOTAL)
    gv = glob.rearrange("p (t two) -> p two t", two=2)
    sums, sqs = gv[:, 0, :], gv[:, 1, :]
    m = small.tile([128, 4], F32, tag=f"m_{name}", name=f"m_{name}")
    nc.vector.tensor_scalar(out=m, in0=sums, scalar1=inv_n, scalar2=None,
                            op0=ALU.mult)
    msq = small.tile([128, 4], F32, tag=f"gmsq_{name}", name=f"gmsq_{name}")
    nc.vector.tensor_mul(out=msq, in0=m, in1=m)
    var = small.tile([128, 4], F32, tag=f"var_{name}", name=f"var_{name}")
    nc.vector.scalar_tensor_tensor(
        out=var, in0=sqs, scalar=inv_n, in1=msq, op0=ALU.mult, op1=ALU.subtract)
    rstd = _rstd_rsqrt(nc, small, name, var, 4)
    nc.vector.tensor_mul(out=s_all, in0=g_sb, in1=rstd)
    sn = small.tile([128, 4], F32, tag=f"sn_{name}", name=f"sn_{name}")
    nc.vector.tensor_mul(out=sn, in0=s_all, in1=m)
    nc.vector.tensor_sub(out=t_all, in0=be_sb, in1=sn)
    return s_all, t_all


def build(sim=False, collectives=True, n_devices=N_CORES, stop_after=None):
    nc = _build_graph(sim=sim, collectives=collectives, n_devices=n_devices,
                      stop_after=stop_after)
    nc.compile()
    return nc


def _build_graph(sim=False, collectives=True, n_devices=N_CORES, stop_after=None):
    from contextlib import ExitStack

    nc = bacc.Bacc("TRN2", target_bir_lowering=False, debug=False,
                   num_devices=n_devices)

    XTB = nc.dram_tensor("xtb", [NT, 128, N_TOTAL], BF16, kind="ExternalInput")
    XT = nc.dram_tensor("xt", [NT, 128, TOK], BF16, kind="ExternalInput")
    WQ = nc.dram_tensor("wq", [128, NT, 512], BF16, kind="ExternalInput")
    WK = nc.dram_tensor("wk", [128, NT, 512], BF16, kind="ExternalInput")
    WV = nc.dram_tensor("wv", [128, NT, 512], BF16, kind="ExternalInput")
    WO = nc.dram_tensor("wo", [128, H // 2, 512], BF16, kind="ExternalInput")
    WM1 = nc.dram_tensor("wm1", [128, NT, 2048], BF16, kind="ExternalInput")
    WM2 = nc.dram_tensor("wm2", [128, NH, 512], BF16, kind="ExternalInput")
    BQ = nc.dram_tensor("bq", [128, 4], F32, kind="ExternalInput")
    BK = nc.dram_tensor("bk", [128, 4], F32, kind="ExternalInput")
    BO2 = nc.dram_tensor("bo2", [128, 4], F32, kind="ExternalInput")
    B1 = nc.dram_tensor("b1m", [128, 16], F32, kind="ExternalInput")
    B2 = nc.dram_tensor("b2m", [128, 4], F32, kind="ExternalInput")
    G1 = nc.dram_tensor("g1", [128, 4], F32, kind="ExternalInput")
    BE1 = nc.dram_tensor("be1", [128, 4], F32, kind="ExternalInput")
    G2 = nc.dram_tensor("g2", [128, 4], F32, kind="ExternalInput")
    BE2 = nc.dram_tensor("be2", [128, 4], F32, kind="ExternalInput")
    AM = nc.dram_tensor("am", [128, 8], F32, kind="ExternalInput")
    OUT = nc.dram_tensor("outt", [NT, 128, TOK], F32, kind="ExternalOutput")

    gelu_func = AF.Exp if sim else AF.Gelu

    with tile.TileContext(nc) as tc, ExitStack() as ctx:
        vec = ctx.enter_context(tc.tile_pool(name="vec", bufs=1))
        small = ctx.enter_context(tc.tile_pool(name="small", bufs=8))
        statsp = ctx.enter_context(tc.tile_pool(name="stats", bufs=1))
        dram = ctx.enter_context(tc.tile_pool(name="dram", bufs=1, space="DRAM"))
        x2p = ctx.enter_context(tc.tile_pool(name="x2", bufs=1))

        # ---- throwaway AllReduce: absorbs the ~65us first-collective
        # warmup concurrently with input DMA + BN1 stats. Result unused.
        if collectives:
            zz = vec.tile([128, 8], F32, tag="zz", name="zz")
            nc.vector.memset(zz, 0.0)
            dum_in = dram.tile([128, 8], F32, tag="dum_i", name="dum_i")
            dum_out = dram.tile([128, 8], F32, tag="dum_o", name="dum_o")
            nc.scalar.dma_start(out=dum_in, in_=zz)
            nc.gpsimd.collective_compute(
                "AllReduce", ALU.add,
                replica_groups=[list(range(N_CORES))],
                ins=[dum_in[:]], outs=[dum_out[:]])
            # result intentionally never read back — a read-back DMA would
            # block its engine queue until the AR completes (~90us)

        def vload(name, dram_t, shape, dtype=F32):
            t = vec.tile(shape, dtype, tag=name, name=name)
            nc.sync.dma_start(out=t, in_=dram_t[:, :])
            return t

        x2_tiles = [x2p.tile([128, TOK], F32, tag=f"x2_{t}", name=f"x2_{t}")
                    for t in range(NT)]
        # opened before s1 so it outlives s1 without breaking pool LIFO
        # order; tiles + DMAs are issued after the BN1 stats scope closes.
        mlpwp = ctx.enter_context(tc.tile_pool(name="mlpw", bufs=1))

        def dump_out(tiles, cast=False):
            for t in range(NT):
                src_ap = tiles[t].bitcast(F32) if cast else tiles[t]
                nc.sync.dma_start(out=OUT[t, :, :], in_=src_ap)

        with ExitStack() as s1:
            xp = s1.enter_context(tc.tile_pool(name="xt", bufs=1))

            # ---- BN1: replicated global stats from the full bf16 x.
            # xtb DMAs are issued FIRST — they gate everything else.
            s1v = statsp.tile([128, 4], F32, tag="s1v", name="s1v")
            t1v = statsp.tile([128, 4], F32, tag="t1v", name="t1v")
            with ExitStack() as sb_scope:
                xbp = sb_scope.enter_context(tc.tile_pool(name="xtb", bufs=1))
                NG = N_TOTAL // 512   # bn_stats free dim is capped at 512
                HALF = N_TOTAL // 2
                xb_tiles = []
                for t in range(NT):
                    xb = xbp.tile([128, N_TOTAL], BF16, tag=f"xb_{t}",
                                  name=f"xb_{t}")
                    # first tile in quarters so its bn_stats start sooner
                    nsplit = 4 if t == 0 else 2
                    step = N_TOTAL // nsplit
                    for s in range(nsplit):
                        nc.sync.dma_start(
                            out=xb[:, s * step : (s + 1) * step],
                            in_=XTB[t, :, s * step : (s + 1) * step])
                    xb_tiles.append(xb)

                # local shard (bf16) — residual + h1 source
                x_tiles = []
                for t in range(NT):
                    xt = xp.tile([128, TOK], BF16, tag=f"x_{t}", name=f"x_{t}")
                    nc.sync.dma_start(out=xt, in_=XT[t, :, :])
                    x_tiles.append(xt)

                g1_sb = vload("g1", G1, [128, 4])
                be1_sb = vload("be1", BE1, [128, 4])
                am_sb = vload("am", AM, [128, 8])
                bq_sb = vload("bq", BQ, [128, 4])
                bk_sb = vload("bk", BK, [128, 4])
                bo2_sb = vload("bo2", BO2, [128, 4])
                g2_sb = vload("g2", G2, [128, 4])
                be2_sb = vload("be2", BE2, [128, 4])
                b1_sb = vload("b1", B1, [128, 16])
                b2_sb = vload("b2", B2, [128, 4])
                eps_sb = vec.tile([128, 1], F32, tag="eps", name="eps")
                nc.vector.memset(eps_sb, EPS)

                # per-tile stats -> mv4[:, t, :] = (mean, var); single batched
                # Ln/Exp for the rstd of all 4 tiles (avoids ACT table thrash)
                mv4 = statsp.tile([128, 4, 2], F32, tag="mv4", name="mv4")
                for t in range(NT):
                    st = small.tile([128, NG, 6], F32, tag=f"s1s_{t}",
                                    name=f"s1s_{t}")
                    for g in range(NG):
                        nc.vector.bn_stats(
                            out=st[:, g, :],
                            in_=xb_tiles[t][:, g * 512 : (g + 1) * 512])
                    nc.vector.bn_aggr(out=mv4[:, t, :], in_=st)
                mean4, var4 = mv4[:, :, 0], mv4[:, :, 1]
                rstd4 = _rstd_rsqrt(nc, small, "bn1", var4, 4)
                nc.vector.tensor_mul(out=s1v, in0=g1_sb, in1=rstd4)
                sn1 = small.tile([128, 4], F32, tag="s1n", name="s1n")
                nc.vector.tensor_mul(out=sn1, in0=s1v, in1=mean4)
                nc.vector.tensor_sub(out=t1v, in0=be1_sb, in1=sn1)
            if stop_after == "bn1":
                dump_out(x2_tiles)
                return nc

            # MLP weights (issued after the xtb pool closes; needed ~120us in)
            wm1_sb = mlpwp.tile([128, NT, 2048], BF16, tag="wm1", name="wm1s")
            nc.sync.dma_start(out=wm1_sb, in_=WM1[:, :, :])
            wm2_sb = mlpwp.tile([128, NH, 512], BF16, tag="wm2", name="wm2s")
            nc.sync.dma_start(out=wm2_sb, in_=WM2[:, :, :])
            wop = s1.enter_context(tc.tile_pool(name="wo", bufs=1))
            wo_sb = wop.tile([128, H // 2, 512], BF16, tag="wo", name="wos")
            nc.sync.dma_start(out=wo_sb, in_=WO[:, :, :])

            qkp = s1.enter_context(tc.tile_pool(name="qk", bufs=1))
            vap = s1.enter_context(tc.tile_pool(name="vaug", bufs=1))
            q_tiles = [qkp.tile([128, TOK], BF16, tag=f"q_{t}", name=f"q_{t}")
                       for t in range(NT)]
            k_tiles = [qkp.tile([128, TOK], BF16, tag=f"k_{t}", name=f"k_{t}")
                       for t in range(NT)]
            v_aug = vap.tile([128, NC_TOK, H, DH + 1], BF16, tag="vaug",
                             name="vaug")
            ones_sb = vec.tile([128, NC_TOK, H, 1], F32, tag="ones",
                               name="ones")
            nc.vector.memset(ones_sb, 1.0)
            nc.vector.tensor_copy(out=v_aug[:, :, :, DH : DH + 1], in_=ones_sb)

            # ======== Phase 1: h1 + QKV projections (bf16) ========
            with ExitStack() as s2:
                wqp = s2.enter_context(tc.tile_pool(name="wqkv", bufs=1))
                h1p = s2.enter_context(tc.tile_pool(name="h1", bufs=1))
                pj = s2.enter_context(
                    tc.tile_pool(name="pj", bufs=3, space="PSUM"))

                wq_sb = wqp.tile([128, NT, 512], BF16, tag="wq", name="wqs")
                nc.sync.dma_start(out=wq_sb, in_=WQ[:, :, :])
                wk_sb = wqp.tile([128, NT, 512], BF16, tag="wk", name="wks")
                nc.sync.dma_start(out=wk_sb, in_=WK[:, :, :])
                wv_sb = wqp.tile([128, NT, 512], BF16, tag="wv", name="wvs")
                nc.sync.dma_start(out=wv_sb, in_=WV[:, :, :])

                h1_tiles = []
                for t in range(NT):
                    h1 = h1p.tile([128, TOK], BF16, tag=f"h1_{t}",
                                  name=f"h1_{t}")
                    nc.vector.tensor_scalar(
                        out=h1, in0=x_tiles[t],
                        scalar1=s1v[:, t : t + 1], scalar2=t1v[:, t : t + 1],
                        op0=ALU.mult, op1=ALU.add)
                    h1_tiles.append(h1)

                # q/k for tile 0 first so head 0 can start early, then v,
                # then the remaining q/k tiles.
                def qk_tile(w_sb, b_sb, dst, o):
                    for hf in range(2):
                        p = pj.tile([128, 512], F32, tag="pjq", name="pjq")
                        for k in range(NT):
                            nc.tensor.matmul(
                                p,
                                w_sb[:, k, o * 128 : (o + 1) * 128],
                                h1_tiles[k][:, hf * 512 : (hf + 1) * 512],
                                start=(k == 0), stop=(k == NT - 1))
                        nc.scalar.activation(
                            out=dst[o][:, hf * 512 : (hf + 1) * 512],
                            in_=p, func=AF.Identity,
                            bias=b_sb[:, o : o + 1], scale=1.0)

                qk_tile(wq_sb, bq_sb, q_tiles, 0)
                qk_tile(wk_sb, bk_sb, k_tiles, 0)

                # v natural: out [tok-chunk 128, 512 feat] -> v_aug (no bias;
                # bv is folded into bo2 host-side)
                for tt in range(NC_TOK):
                    p = pj.tile([128, 512], F32, tag="pjv", name="pjv")
                    for k in range(NT):
                        nc.tensor.matmul(
                            p,
                            h1_tiles[k][:, tt * 128 : (tt + 1) * 128],
                            wv_sb[:, k, :],
                            start=(k == 0), stop=(k == NT - 1))
                    nc.vector.tensor_copy(
                        out=v_aug[:, tt, :, 0:DH],
                        in_=p.rearrange("p (h d) -> p h d", h=H))

                for o in range(1, NT):
                    qk_tile(wq_sb, bq_sb, q_tiles, o)
                    qk_tile(wk_sb, bk_sb, k_tiles, o)

            if stop_after == "qkv":
                dump_out(q_tiles, cast=False)
                return nc

            # ======== Phase 2: attention ========
            otp = s1.enter_context(tc.tile_pool(name="ot", bufs=1))
            # row 64 carries the softmax denominators (one column block per
            # head); a single SBUF->SBUF DMA regathers them as [8, TOK].
            oT = otp.tile([DH + 1, H, TOK], F32, tag="ot", name="ots")
            # odd heads' unnormalized data DMA-shifted to partitions 64-127
            oTs = otp.tile([128, H // 2, TOK], F32, tag="otsh", name="otshs")
            # normalized head-pair blocks (even rows 0-63, odd rows 64-127)
            oTp = otp.tile([128, H // 2, TOK], BF16, tag="otp", name="otps")
            rcp_f = otp.tile([1, H, TOK], BF16, tag="rcpf", name="rcpfs")
            ones8f = vec.tile([1, 64], F32, tag="ones8f", name="ones8f")
            nc.vector.memset(ones8f, 1.0)
            ones8 = vec.tile([1, 64], BF16, tag="ones8", name="ones8")
            nc.vector.tensor_copy(out=ones8, in_=ones8f)

            dnp = s1.enter_context(tc.tile_pool(name="dn", bufs=1))

            def denom_batch(lo, hi):
                """1/denom for heads [lo, hi) on the DVE (approx ~2 ULP) —
                keeps ScalarE free for exps, no ACT table switches. Engine
                ops need base partition 0; both batches share one tile set
                (batch 1's values are already flattened into rcp_f)."""
                n = hi - lo
                den = dnp.tile([n, TOK], F32, tag="den", name=f"den{lo}")
                scr = dnp.tile([n, TOK], F32, tag="dsc", name=f"dsc{lo}")
                r32 = dnp.tile([n, TOK], F32, tag="r32", name=f"r32_{lo}")
                rb = dnp.tile([n, TOK], BF16, tag="rb", name=f"rb{lo}")
                nc.gpsimd.dma_start(
                    out=den,
                    in_=oT[64:65, lo:hi, :].rearrange("p h q -> p (h q)"))
                nc.vector.reciprocal_approx_accurate(
                    out=r32, in_=den, scratch=scr)
                nc.vector.tensor_copy(out=rb, in_=r32)
                nc.gpsimd.dma_start(out=rcp_f[0:1, lo:hi, :], in_=rb)

            with ExitStack() as s3:
                scp = s3.enter_context(
                    tc.tile_pool(name="sc", bufs=1, space="PSUM"))
                avp = s3.enter_context(
                    tc.tile_pool(name="av", bufs=1, space="PSUM"))
                ep = s3.enter_context(tc.tile_pool(name="E", bufs=2))

                # head PAIRS: heads 2p (q/k rows 0:64) and 2p+1 (rows 64:128)
                # of q/k tile p. The two score matmuls use disjoint PE row
                # strips (base partitions 0 / 64) and run concurrently.
                for p in range(H // 2):
                    he, ho = 2 * p, 2 * p + 1
                    av_e = avp.tile([DH + 1, TOK], F32, tag="av_e",
                                    name="av_es")
                    av_o = avp.tile([DH + 1, TOK], F32, tag="av_o",
                                    name="av_os")

                    def av_mm(av, head, pc, pe):
                        for hf in range(2):
                            nc.tensor.matmul(
                                av[:, hf * 512 : (hf + 1) * 512],
                                v_aug[:, pc, head, :],
                                pe[:, hf * 512 : (hf + 1) * 512],
                                start=(pc == 0), stop=(pc == NC_TOK - 1))

                    prev = None
                    for c in range(NC_TOK):
                        S_e = scp.tile([128, TOK], F32, tag="Se", name="Ses")
                        S_o = scp.tile([128, TOK], F32, tag="So", name="Sos")
                        for hf in range(2):
                            nc.tensor.matmul(
                                S_e[:, hf * 512 : (hf + 1) * 512],
                                k_tiles[p][0:64, c * 128 : (c + 1) * 128],
                                q_tiles[p][0:64, hf * 512 : (hf + 1) * 512],
                                start=True, stop=True)
                            nc.tensor.matmul(
                                S_o[:, hf * 512 : (hf + 1) * 512],
                                k_tiles[p][64:128, c * 128 : (c + 1) * 128],
                                q_tiles[p][64:128, hf * 512 : (hf + 1) * 512],
                                start=True, stop=True)
                        E_e = ep.tile([128, TOK], BF16, tag="Ee", name="Ees")
                        nc.scalar.activation(
                            out=E_e, in_=S_e, func=AF.Exp,
                            bias=am_sb[:, c : c + 1], scale=0.125)
                        E_o = ep.tile([128, TOK], BF16, tag="Eo", name="Eos")
                        nc.scalar.activation(
                            out=E_o, in_=S_o, func=AF.Exp,
                            bias=am_sb[:, c : c + 1], scale=0.125)
                        if prev is not None:
                            pc, pe_e, pe_o = prev
                            av_mm(av_e, he, pc, pe_e)
                            av_mm(av_o, ho, pc, pe_o)
                        prev = (c, E_e, E_o)
                    pc, pe_e, pe_o = prev
                    av_mm(av_e, he, pc, pe_e)
                    av_mm(av_o, ho, pc, pe_o)
                    # evacuate unnormalized output + denominator row together
                    nc.vector.tensor_copy(out=oT[:, he, :], in_=av_e[:, :])
                    nc.vector.tensor_copy(out=oT[:, ho, :], in_=av_o[:, :])
                    # odd head's data shifted to partitions 64-127 for the
                    # pair-packed (K=128) output projection
                    nc.sync.dma_start(out=oTs[64:128, p, :],
                                      in_=oT[0:64, ho, :])
                    if p == 1:
                        denom_batch(0, 4)
                denom_batch(4, 8)
                # prewarm the sqrt table for BN2 while PE does the O-proj
                # (input den_sb pins this after the attention exps)
                warm_rs = vec.tile([1, 1], F32, tag="warm_rs", name="warm_rs")
                nc.scalar.activation(out=warm_rs, in_=rcp_f[0:1, 7, 0:1],
                                     func=AF.Sqrt)

            if stop_after == "attn":
                dump_out(x2_tiles)
                return nc

            # ======== Phase 3: normalize + output projection ========
            with ExitStack() as s4:
                bcp = s4.enter_context(
                    tc.tile_pool(name="bc", bufs=2, space="PSUM"))
                pop = s4.enter_context(
                    tc.tile_pool(name="po", bufs=4, space="PSUM"))
                rp = s4.enter_context(tc.tile_pool(name="rcp", bufs=1))

                for p in range(H // 2):
                    he, ho = 2 * p, 2 * p + 1
                    bc = bcp.tile([128, TOK], F32, tag="bc", name="bcs")
                    for hf in range(2):
                        nc.tensor.matmul(
                            bc[0:64, hf * 512 : (hf + 1) * 512],
                            ones8[0:1, :],
                            rcp_f[0:1, he, hf * 512 : (hf + 1) * 512],
                            start=True, stop=True)
                        nc.tensor.matmul(
                            bc[64:128, hf * 512 : (hf + 1) * 512],
                            ones8[0:1, :],
                            rcp_f[0:1, ho, hf * 512 : (hf + 1) * 512],
                            start=True, stop=True)
                    nc.vector.tensor_mul(
                        out=oTp[0:64, p, :], in0=oT[0:64, he, :],
                        in1=bc[0:64, :])
                    nc.vector.tensor_mul(
                        out=oTp[64:128, p, :], in0=oTs[64:128, p, :],
                        in1=bc[64:128, :])

                bn2_local = statsp.tile([128, 8], F32, tag="loc_bn2",
                                        name="loc_bn2")
                for o in range(NT):
                    for hf in range(2):
                        p = pop.tile([128, 512], F32, tag="po", name="pos")
                        for pr in range(H // 2):
                            nc.tensor.matmul(
                                p,
                                wo_sb[:, pr, o * 128 : (o + 1) * 128],
                                oTp[:, pr, hf * 512 : (hf + 1) * 512],
                                start=(pr == 0), stop=(pr == H // 2 - 1))
                        # x2 = (proj + bo2) + x
                        nc.vector.scalar_tensor_tensor(
                            out=x2_tiles[o][:, hf * 512 : (hf + 1) * 512],
                            in0=p, scalar=bo2_sb[:, o : o + 1],
                            in1=x_tiles[o][:, hf * 512 : (hf + 1) * 512],
                            op0=ALU.add, op1=ALU.add)
                    _bn_partial(nc, small, "bn2", o, x2_tiles[o], bn2_local)

        if stop_after == "oproj":
            dump_out(x2_tiles)
            return nc

        # ======== Phase 4: BN2 + MLP ========
        with ExitStack() as s5:
            h2p = s5.enter_context(tc.tile_pool(name="h2", bufs=1))
            htp = s5.enter_context(tc.tile_pool(name="ht", bufs=1))
            outp = s5.enter_context(tc.tile_pool(name="outsb", bufs=2))
            pm1 = s5.enter_context(
                tc.tile_pool(name="pm1", bufs=2, space="PSUM"))
            pm2 = s5.enter_context(
                tc.tile_pool(name="pm2", bufs=4, space="PSUM"))

            s2v, t2v = _bn_device(nc, (small, statsp, dram),
                                  g2_sb, be2_sb, "bn2",
                                  collectives=collectives, local=bn2_local)
            # prewarm the gelu table; input s2v pins this AFTER bn2's ln/exp
            # so the load lands in the h2-cast window, not at kernel start
            warm3 = vec.tile([128, 1], F32, tag="warm3", name="warm3")
            nc.scalar.activation(out=warm3, in_=s2v[:, 0:1], func=gelu_func)

            h2_tiles = []
            for t in range(NT):
                h2 = h2p.tile([128, TOK], BF16, tag=f"h2_{t}", name=f"h2_{t}")
                nc.vector.tensor_scalar(
                    out=h2, in0=x2_tiles[t],
                    scalar1=s2v[:, t : t + 1], scalar2=t2v[:, t : t + 1],
                    op0=ALU.mult, op1=ALU.add)
                h2_tiles.append(h2)

            ht = htp.tile([128, NH, TOK], BF16, tag="ht", name="hts")
            for o in range(NH):
                p = pm1.tile([128, TOK], F32, tag="pm1", name="pm1s")
                for hf in range(2):
                    for k in range(NT):
                        nc.tensor.matmul(
                            p[:, hf * 512 : (hf + 1) * 512],
                            wm1_sb[:, k, o * 128 : (o + 1) * 128],
                            h2_tiles[k][:, hf * 512 : (hf + 1) * 512],
                            start=(k == 0), stop=(k == NT - 1))
                nc.scalar.activation(
                    out=ht[:, o, :], in_=p, func=gelu_func,
                    bias=b1_sb[:, o : o + 1], scale=1.0)

            for o in range(NT):
                ot = outp.tile([128, TOK], F32, tag="osb", name="osbs")
                for hf in range(2):
                    p = pm2.tile([128, 512], F32, tag="pm2", name="pm2s")
                    for k in range(NH):
                        nc.tensor.matmul(
                            p,
                            wm2_sb[:, k, o * 128 : (o + 1) * 128],
                            ht[:, k, hf * 512 : (hf + 1) * 512],
                            start=(k == 0), stop=(k == NH - 1))
                    nc.vector.scalar_tensor_tensor(
                        out=ot[:, hf * 512 : (hf + 1) * 512],
                        in0=p, scalar=b2_sb[:, o : o + 1],
                        in1=x2_tiles[o][:, hf * 512 : (hf + 1) * 512],
                        op0=ALU.add, op1=ALU.add)
                # gpsimd queue: idle after the BN2 AR — the sync queue's
                # head is blocked ~90us on an earlier semaphore
                nc.gpsimd.dma_start(out=OUT[o, :, :], in_=ot)

    return nc


_NC_CACHE = {}


def _get_nc(sim=False):
    if sim not in _NC_CACHE:
        _NC_CACHE[sim] = build(sim=sim)
    return _NC_CACHE[sim]


def make_in_maps(x, mask, Wq, bq, Wk, bk, Wv, bv, Wo, bo, g1, be1, g2, be2,
                 W1, b1m, W2, b2m):
    """Host-side sharding + layout prep. Returns list of per-core input dicts."""
    bf16 = ml_dtypes.bfloat16
    xT = np.ascontiguousarray(np.asarray(x, np.float32).T)      # [512, 8192]
    xTb = np.ascontiguousarray(
        xT.reshape(NT, 128, N_TOTAL)).astype(bf16)

    def wprep(W, nt):
        return np.ascontiguousarray(
            np.asarray(W, np.float32).reshape(nt, 128, -1).transpose(1, 0, 2)
        ).astype(bf16)

    wq = wprep(Wq, NT)
    wk = wprep(Wk, NT)
    wv = wprep(Wv, NT)
    # head-pair blocks: row r of pair p = Wo row p*128 + r
    wo = np.ascontiguousarray(
        np.asarray(Wo, np.float32).reshape(H // 2, 128, 512).transpose(1, 0, 2)
    ).astype(bf16)
    wm1 = wprep(W1, NT)
    wm2 = wprep(W2, NH)

    def pp(v, c):
        return np.ascontiguousarray(np.asarray(v, np.float32).reshape(c, 128).T)

    bo2 = np.asarray(bo, np.float32) + (
        np.asarray(bv, np.float32) @ np.asarray(Wo, np.float32))

    shared = {
        "xtb": xTb,
        "wq": wq, "wk": wk, "wv": wv, "wo": wo, "wm1": wm1, "wm2": wm2,
        "bq": pp(bq, 4), "bk": pp(bk, 4), "bo2": pp(bo2, 4),
        "b1m": pp(b1m, 16), "b2m": pp(b2m, 4),
        "g1": pp(g1, 4), "be1": pp(be1, 4), "g2": pp(g2, 4), "be2": pp(be2, 4),
    }
    am_full = np.where(np.asarray(mask, bool), 0.0, -1e9).astype(np.float32)
    in_maps = []
    for core in range(N_CORES):
        sl = xT[:, core * TOK : (core + 1) * TOK]
        m = dict(shared)
        m["xt"] = np.ascontiguousarray(sl.reshape(NT, 128, TOK)).astype(bf16)
        m["am"] = np.ascontiguousarray(am_full[core].reshape(8, 128).T)
        in_maps.append(m)
    return in_maps


_EXEC_CACHE = {}


def _get_executor():
    """Cached PJRT executor for the compiled kernel (same path
    run_bass_kernel_spmd takes under axon, but jitted once and reused)."""
    if "fn" in _EXEC_CACHE:
        return _EXEC_CACHE["fn"]
    import jax
    from jax.sharding import Mesh, PartitionSpec
    from jax.experimental.shard_map import shard_map
    import concourse.bass2jax as b2j

    nc = _get_nc(sim=False)
    b2j.install_neuronx_cc_hook()
    partition_name = (nc.partition_id_tensor.name
                      if nc.partition_id_tensor else None)
    in_names, out_names, out_avals, zero_outs = [], [], [], []
    for alloc in nc.m.functions[0].allocations:
        if not isinstance(alloc, mybir.MemoryLocationSet):
            continue
        name = alloc.memorylocations[0].name
        if alloc.kind == "ExternalInput":
            if name != partition_name:
                in_names.append(name)
        elif alloc.kind == "ExternalOutput":
            out_names.append(name)
            shape = tuple(alloc.tensor_shape)
            dtype = mybir.dt.np(alloc.dtype)
            out_avals.append(jax.core.ShapedArray(shape, dtype))
            zero_outs.append(np.zeros(shape, dtype))
    n_params = len(in_names)
    all_names = in_names + out_names
    if partition_name is not None:
        all_names = all_names + [partition_name]

    def _body(*args):
        operands = list(args)
        if partition_name is not None:
            operands.append(b2j.partition_id_tensor())
        return tuple(b2j._bass_exec_p.bind(
            *operands,
            out_avals=tuple(out_avals),
            in_names=tuple(all_names),
            out_names=tuple(out_names),
            lowering_input_output_aliases=(),
            sim_require_finite=True,
            sim_require_nnan=True,
            nc=nc,
        ))

    devices = jax.devices()[:N_CORES]
    mesh = Mesh(np.asarray(devices), ("core",))
    n_out = len(out_names)
    sharded = jax.jit(
        shard_map(_body, mesh=mesh,
                  in_specs=(PartitionSpec("core"),) * (n_params + n_out),
                  out_specs=(PartitionSpec("core"),) * n_out,
                  check_rep=False),
        keep_unused=True)

    def run(in_maps):
        per_core = [[np.asarray(m[nm]) for nm in in_names] for m in in_maps]
        concat_in = [
            np.concatenate([per_core[c][i] for c in range(N_CORES)], axis=0)
            for i in range(n_params)]
        concat_zeros = [
            np.zeros((N_CORES * z.shape[0], *z.shape[1:]), z.dtype)
            for z in zero_outs]
        out_arrs = sharded(*concat_in, *concat_zeros)
        return [
            {name: np.asarray(out_arrs[i]).reshape(
                N_CORES, *out_avals[i].shape)[c]
             for i, name in enumerate(out_names)}
            for c in range(N_CORES)]

    _EXEC_CACHE["fn"] = run
    return run


def gather_out(results):
    """results: list of per-core dicts with 'outt' [4, 128, 1024] -> [8192, 512]."""
    outs = []
    for core in range(N_CORES):
        oT = results[core]["outt"].reshape(ED, TOK)   # [512, 1024]
        outs.append(oT.T)                             # [1024, 512]
    return np.concatenate(outs, axis=0).astype(np.float32)


def kernel(**inputs) -> np.ndarray:
    inputs = dict(inputs)
    inputs.pop("b", None)
    inputs.pop("gs", None)
    in_maps = make_in_maps(**inputs)
    run = _get_executor()
    return gather_out(run(in_maps))
